# revision 5
# baseline (speedup 1.0000x reference)
"""DynamicSparseMoE Trainium2 kernel.

Math (per token t):
  logits[e'] = x[t] . gate_w[e'] + gate_b[e']        (C=2048 contraction)
  gw[e']     = 1.0 if logits[e'] > 0 else 0.0
  expert e input: xe[d] = x[t, 16*d + e]  (d=0..127; expert idx fastest in channel)
  h  = gelu(fc_w[e] @ xe + fc_b[e])                   (H=512)
  oe = proj_w[e] @ h + proj_b[e]                      (DE=128)
  out[t, 128*e + d] = gw[e] * oe[d]                   (expert-major output channels)

Strategy: data-parallel over the 16384 tokens across 8 NeuronCores (2048
tokens/core).  Per 512-token group:
  pass1 (per expert): 4 PE transposes of the stride-16 channel slice of the
    row-major x tile -> xe^T [de, tok] fp32; DVE evacuation; 4 exact-fp32
    gate matmuls (activation-stationary, slice-accumulated into a single
    PSUM bank); GPSIMD cast to fp32r; fc as fp32r matmuls (weights
    stationary, N=512); gelu+fc_bias fused on ACT writing fp32r; proj as
    fp32r matmuls accumulating K=512; proj_bias fused into the PSUM
    evacuation (bf16 out).
  pass2: gate threshold (is_gt) on DVE.
  pass3 (per expert): 4 bf16 PE exit transposes; gate multiply fused into
    the final PSUM->SBUF copy as a per-token tensor_scalar; contiguous
    row stores.
"""

import sys

for _p in ("/opt/trn_rl_repo", "/root/.axon_site"):
    if _p not in sys.path:
        sys.path.insert(0, _p)

import ml_dtypes
import numpy as np

import concourse.mybir as mybir
from concourse import bacc
from concourse.bass_utils import run_bass_kernel_spmd
from concourse.tile import TileContext

B, T, C, E = 8, 2048, 2048, 16
DE = C // E  # 128
H = 4 * DE  # 512
NCORES = 8
NTOK = B * T  # 16384
TPC = NTOK // NCORES  # tokens per core: 2048
GROUP = 512  # tokens per group
NTAU = GROUP // 128  # 4 token-tiles per group
NGRP = TPC // GROUP  # 4 groups per core

F32 = mybir.dt.float32
F32R = mybir.dt.float32r
BF16 = mybir.dt.bfloat16
AF = mybir.ActivationFunctionType
ALU = mybir.AluOpType
GELU = AF.Gelu

_CACHE = {}


def _build():
    nc = bacc.Bacc(trn_type="TRN2", num_devices=NCORES)

    x_d = nc.dram_tensor("x", [TPC, C], F32, kind="ExternalInput").ap()
    gwp_d = nc.dram_tensor("gwp", [C, E], F32, kind="ExternalInput").ap()
    fcw_d = nc.dram_tensor("fcw", [E, DE, H], F32, kind="ExternalInput").ap()
    pjw_d = nc.dram_tensor("pjw", [E, 4, 128, DE], F32, kind="ExternalInput").ap()
    fcb_d = nc.dram_tensor("fcb", [128, 64], F32, kind="ExternalInput").ap()
    pjb_d = nc.dram_tensor("pjb", [128, E], F32, kind="ExternalInput").ap()
    ngb_d = nc.dram_tensor("ngb", [128, E], F32, kind="ExternalInput").ap()
    idn_d = nc.dram_tensor("idn", [128, 128], F32, kind="ExternalInput").ap()
    idnb_d = nc.dram_tensor("idnb", [128, 128], BF16, kind="ExternalInput").ap()
    out_d = nc.dram_tensor("out", [TPC, C], F32, kind="ExternalOutput").ap()

    with TileContext(nc) as tc:
        with (
            tc.tile_pool(name="wts", bufs=1) as wts,
            tc.tile_pool(name="work", bufs=2) as work,
            tc.tile_pool(name="psum", bufs=2, space="PSUM") as psum,
        ):
            # ---- resident weights ----
            gwp_sb = wts.tile([128, E * E], F32)  # [p, chunk*16+e']
            nc.sync.dma_start(
                out=gwp_sb.rearrange("p (k e) -> p k e", k=E),
                in_=gwp_d.rearrange("(k p) e -> p k e", p=128),
            )
            # fc/proj weights: SWDGE dma with cast fp32 -> fp32r
            fcw_sb = wts.tile([128, E * H], F32R)  # [de, e*512+h]
            nc.gpsimd.dma_start(
                out=fcw_sb.rearrange("p (e h) -> p e h", e=E),
                in_=fcw_d.rearrange("e p h -> p e h"),
            )
            pjw_sb = wts.tile([128, E * 4 * DE], F32R)  # [h_in_chunk, (e*4+q)*128+d]
            nc.gpsimd.dma_start(
                out=pjw_sb.rearrange("p (e q d) -> p e q d", e=E, q=4),
                in_=pjw_d.rearrange("e q p d -> p e q d"),
            )
            fcb_sb = wts.tile([128, 64], F32)
            nc.sync.dma_start(out=fcb_sb, in_=fcb_d)
            pjb_sb = wts.tile([128, E], F32)
            nc.sync.dma_start(out=pjb_sb, in_=pjb_d)
            ngb_sb = wts.tile([128, E], F32)
            nc.sync.dma_start(out=ngb_sb, in_=ngb_d)
            idn_sb = wts.tile([128, 128], F32)
            nc.sync.dma_start(out=idn_sb, in_=idn_d)
            idnb_sb = wts.tile([128, 128], BF16)
            nc.sync.dma_start(out=idnb_sb, in_=idnb_d)

            for g in range(NGRP):
                t0 = g * GROUP
                xrow = []
                for ti in range(NTAU):
                    xt = work.tile([128, C], F32, tag="xrow", bufs=4)
                    nc.sync.dma_start(
                        out=xt, in_=x_d[t0 + ti * 128 : t0 + (ti + 1) * 128, :]
                    )
                    xrow.append(xt)

                ps_g = psum.tile([128, NTAU * E], F32, tag="gate", bufs=1)
                xpTr = []
                pjT = []
                # ---- pass 1: per-expert transposes, gate, fc, gelu, proj ----
                for e in range(E):
                    ps_t = psum.tile([128, GROUP], F32, tag="tp", bufs=3)
                    for ti in range(NTAU):
                        lhs = xrow[ti].rearrange("p (d e) -> p e d", e=E)[:, e, :]
                        nc.tensor.transpose(
                            ps_t[:, ti * 128 : (ti + 1) * 128], lhs, idn_sb
                        )
                    xe = work.tile([128, GROUP], F32, tag="xpT", bufs=4)
                    nc.vector.tensor_copy(xe, ps_t)
                    # gate: exact fp32, activation-stationary, one PSUM bank
                    for ti in range(NTAU):
                        nc.tensor.matmul(
                            ps_g[:, ti * E : (ti + 1) * E],
                            lhsT=xe[:, ti * 128 : (ti + 1) * 128],
                            rhs=gwp_sb[:, e * E : (e + 1) * E],
                            start=(e == 0 and ti == 0),
                            stop=(e == E - 1 and ti == NTAU - 1),
                            skip_group_check=True,
                        )
                    xer = work.tile([128, GROUP], F32R, tag="xpTr", bufs=3)
                    nc.gpsimd.tensor_copy(xer, xe)
                    xpTr.append(xer)

                    h_sb = work.tile([128, 4 * GROUP], F32R, tag="h", bufs=2)
                    for hq in range(4):
                        ps_fc = psum.tile([128, GROUP], F32, tag="fc", bufs=2)
                        nc.tensor.matmul(
                            ps_fc,
                            lhsT=fcw_sb[:, e * H + hq * 128 : e * H + (hq + 1) * 128],
                            rhs=xer,
                            start=True,
                            stop=True,
                        )
                        nc.scalar.activation(
                            h_sb[:, hq * GROUP : (hq + 1) * GROUP],
                            ps_fc,
                            GELU,
                            bias=fcb_sb[:, e * 4 + hq : e * 4 + hq + 1],
                            scale=1.0,
                        )
                    ps_pj = psum.tile([128, GROUP], F32, tag="pj", bufs=2)
                    for hq in range(4):
                        nc.tensor.matmul(
                            ps_pj,
                            lhsT=pjw_sb[
                                :, (e * 4 + hq) * 128 : (e * 4 + hq + 1) * 128
                            ],
                            rhs=h_sb[:, hq * GROUP : (hq + 1) * GROUP],
                            start=(hq == 0),
                            stop=(hq == 3),
                        )
                    pjT_sb = work.tile([128, GROUP], BF16, tag="pjT", bufs=18)
                    nc.vector.tensor_scalar_add(pjT_sb, ps_pj, pjb_sb[:, e : e + 1])
                    pjT.append(pjT_sb)

                # ---- pass 2: gate threshold ----
                gw = []
                for ti in range(NTAU):
                    gwt = work.tile([128, E], F32, tag="gw", bufs=8)
                    nc.vector.tensor_tensor(
                        gwt, ps_g[:, ti * E : (ti + 1) * E], ngb_sb, ALU.is_gt
                    )
                    gw.append(gwt)

                out_sb = [
                    work.tile([128, C], F32, tag="out", bufs=4, name=f"osb_{g}_{ti}")
                    for ti in range(NTAU)
                ]

                # ---- pass 3: exit transposes + gated evacuation ----
                for e in range(E):
                    ps_o = psum.tile([128, GROUP], BF16, tag="tp", bufs=3)
                    for ti in range(NTAU):
                        nc.tensor.transpose(
                            ps_o[:, ti * 128 : (ti + 1) * 128],
                            pjT[e][:, ti * 128 : (ti + 1) * 128],
                            idnb_sb,
                        )
                    for ti in range(NTAU):
                        nc.vector.tensor_scalar_mul(
                            out_sb[ti][:, e * 128 : (e + 1) * 128],
                            ps_o[:, ti * 128 : (ti + 1) * 128],
                            gw[ti][:, e : e + 1],
                        )

                for ti in range(NTAU):
                    nc.sync.dma_start(
                        out=out_d[t0 + ti * 128 : t0 + (ti + 1) * 128, :],
                        in_=out_sb[ti],
                    )

    nc.compile()
    return nc


def _prep_inputs(x, gate_w, gate_b, fc_w, fc_b, proj_w, proj_b):
    x = np.ascontiguousarray(np.asarray(x, dtype=np.float32)).reshape(NTOK, C)
    gate_w = np.asarray(gate_w, dtype=np.float32)
    gate_b = np.asarray(gate_b, dtype=np.float32)
    fc_w = np.asarray(fc_w, dtype=np.float32)
    fc_b = np.asarray(fc_b, dtype=np.float32)
    proj_w = np.asarray(proj_w, dtype=np.float32)
    proj_b = np.asarray(proj_b, dtype=np.float32)

    # permuted channel order: c' = e*128 + d  ->  orig c = 16*d + e
    cp = np.arange(C)
    orig = 16 * (cp % DE) + cp // DE
    gwp = np.ascontiguousarray(gate_w[:, orig].T)  # [C, E]
    fcw = np.ascontiguousarray(fc_w.transpose(0, 2, 1))  # [E, DE, H]
    pjw = np.ascontiguousarray(
        proj_w.transpose(0, 2, 1).reshape(E, 4, 128, DE)
    )  # [E, q, h_in_chunk, d]
    fcb = np.ascontiguousarray(
        fc_b.reshape(E, 4, 128).transpose(2, 0, 1).reshape(128, E * 4)
    )
    pjb = np.ascontiguousarray(proj_b.T)  # [DE, E]
    ngb = np.ascontiguousarray(np.broadcast_to(-gate_b, (128, E)))
    idn = np.eye(128, dtype=np.float32)
    idnb = np.eye(128, dtype=np.float32).astype(ml_dtypes.bfloat16)

    shared = {
        "gwp": gwp,
        "fcw": fcw,
        "pjw": pjw,
        "fcb": fcb,
        "pjb": pjb,
        "ngb": ngb,
        "idn": idn,
        "idnb": idnb,
    }
    in_maps = [
        {"x": np.ascontiguousarray(x[i * TPC : (i + 1) * TPC]), **shared}
        for i in range(NCORES)
    ]
    return in_maps


def kernel(x, gate_w, gate_b, fc_w, fc_b, proj_w, proj_b, _trace=False, _tmpdir=None):
    if "nc" not in _CACHE:
        _CACHE["nc"] = _build()
    nc = _CACHE["nc"]
    in_maps = _prep_inputs(x, gate_w, gate_b, fc_w, fc_b, proj_w, proj_b)
    res = run_bass_kernel_spmd(
        nc,
        in_maps,
        core_ids=list(range(NCORES)),
        trace=_trace,
        tmpdir=_tmpdir,
    )
    out = np.concatenate([res.results[i]["out"] for i in range(NCORES)], axis=0)
    out = out.reshape(B, T, C)
    if _trace:
        _CACHE["last_result"] = res
    return out


# revision 6
# speedup vs baseline: 1.0220x; 1.0220x over previous
"""DynamicSparseMoE Trainium2 kernel.

Math (per token t):
  logits[e'] = x[t] . gate_w[e'] + gate_b[e']        (C=2048 contraction)
  gw[e']     = 1.0 if logits[e'] > 0 else 0.0
  expert e input: xe[d] = x[t, 16*d + e]  (d=0..127; expert idx fastest in channel)
  h  = gelu(fc_w[e] @ xe + fc_b[e])                   (H=512)
  oe = proj_w[e] @ h + proj_b[e]                      (DE=128)
  out[t, 128*e + d] = gw[e] * oe[d]                   (expert-major output channels)

Strategy: data-parallel over the 16384 tokens across 8 NeuronCores (2048
tokens/core).  Per 512-token group:
  pass1 (per expert): 4 PE transposes of the stride-16 channel slice of the
    row-major x tile -> xe^T [de, tok] fp32; DVE evacuation; 4 exact-fp32
    gate matmuls (activation-stationary, slice-accumulated into a single
    PSUM bank); GPSIMD cast to fp32r; fc as fp32r matmuls (weights
    stationary, N=512); gelu+fc_bias fused on ACT writing fp32r; proj as
    fp32r matmuls accumulating K=512; proj_bias fused into the PSUM
    evacuation (bf16 out).
  pass2: gate threshold (is_gt) on DVE.
  pass3 (per expert): 4 bf16 PE exit transposes; gate multiply fused into
    the final PSUM->SBUF copy as a per-token tensor_scalar; contiguous
    row stores.
"""

import sys

for _p in ("/opt/trn_rl_repo", "/root/.axon_site"):
    if _p not in sys.path:
        sys.path.insert(0, _p)

import ml_dtypes
import numpy as np

import concourse.mybir as mybir
from concourse import bacc
from concourse.bass_utils import run_bass_kernel_spmd
from concourse.tile import TileContext

B, T, C, E = 8, 2048, 2048, 16
DE = C // E  # 128
H = 4 * DE  # 512
NCORES = 8
NTOK = B * T  # 16384
TPC = NTOK // NCORES  # tokens per core: 2048
GROUP = 512  # tokens per group
NTAU = GROUP // 128  # 4 token-tiles per group
NGRP = TPC // GROUP  # 4 groups per core

F32 = mybir.dt.float32
F32R = mybir.dt.float32r
BF16 = mybir.dt.bfloat16
AF = mybir.ActivationFunctionType
ALU = mybir.AluOpType
GELU = AF.Gelu

_CACHE = {}


def _build():
    nc = bacc.Bacc(trn_type="TRN2", num_devices=NCORES)

    x_d = nc.dram_tensor("x", [TPC, C], F32, kind="ExternalInput").ap()
    gwp_d = nc.dram_tensor("gwp", [C, E], F32, kind="ExternalInput").ap()
    fcw_d = nc.dram_tensor("fcw", [E, DE, H], F32, kind="ExternalInput").ap()
    pjw_d = nc.dram_tensor("pjw", [E, 4, 128, DE], F32, kind="ExternalInput").ap()
    fcb_d = nc.dram_tensor("fcb", [128, 64], F32, kind="ExternalInput").ap()
    pjb_d = nc.dram_tensor("pjb", [128, E], F32, kind="ExternalInput").ap()
    ngb_d = nc.dram_tensor("ngb", [128, E], F32, kind="ExternalInput").ap()
    idn_d = nc.dram_tensor("idn", [128, 128], F32, kind="ExternalInput").ap()
    idnb_d = nc.dram_tensor("idnb", [128, 128], BF16, kind="ExternalInput").ap()
    out_d = nc.dram_tensor("out", [TPC, C], F32, kind="ExternalOutput").ap()

    with TileContext(nc) as tc:
        with (
            tc.tile_pool(name="wts", bufs=1) as wts,
            tc.tile_pool(name="work", bufs=2) as work,
            tc.tile_pool(name="psum", bufs=2, space="PSUM") as psum,
        ):
            # ---- resident weights ----
            gwp_sb = wts.tile([128, E * E], F32)  # [p, chunk*16+e']
            nc.sync.dma_start(
                out=gwp_sb.rearrange("p (k e) -> p k e", k=E),
                in_=gwp_d.rearrange("(k p) e -> p k e", p=128),
            )
            # fc/proj weights: SWDGE dma with cast fp32 -> fp32r
            fcw_sb = wts.tile([128, E * H], F32R)  # [de, e*512+h]
            nc.gpsimd.dma_start(
                out=fcw_sb.rearrange("p (e h) -> p e h", e=E),
                in_=fcw_d.rearrange("e p h -> p e h"),
            )
            pjw_sb = wts.tile([128, E * 4 * DE], F32R)  # [h_in_chunk, (e*4+q)*128+d]
            nc.gpsimd.dma_start(
                out=pjw_sb.rearrange("p (e q d) -> p e q d", e=E, q=4),
                in_=pjw_d.rearrange("e q p d -> p e q d"),
            )
            fcb_sb = wts.tile([128, 64], F32)
            nc.sync.dma_start(out=fcb_sb, in_=fcb_d)
            pjb_sb = wts.tile([128, E], F32)
            nc.sync.dma_start(out=pjb_sb, in_=pjb_d)
            ngb_sb = wts.tile([128, E], F32)
            nc.sync.dma_start(out=ngb_sb, in_=ngb_d)
            idn_sb = wts.tile([128, 128], F32)
            nc.sync.dma_start(out=idn_sb, in_=idn_d)
            idnb_sb = wts.tile([128, 128], BF16)
            nc.sync.dma_start(out=idnb_sb, in_=idnb_d)

            for g in range(NGRP):
                t0 = g * GROUP
                xrow = []
                for ti in range(NTAU):
                    xt = work.tile([128, C], F32, tag="xrow", bufs=4)
                    nc.sync.dma_start(
                        out=xt, in_=x_d[t0 + ti * 128 : t0 + (ti + 1) * 128, :]
                    )
                    xrow.append(xt)

                ps_g = psum.tile([128, NTAU * E], F32, tag="gate", bufs=1)
                xpTr = []
                pjT = []
                # ---- pass 1: per-expert transposes, gate, fc, gelu, proj ----
                for e in range(E):
                    ps_t = psum.tile([128, GROUP], F32, tag="tp", bufs=3)
                    for ti in range(NTAU):
                        lhs = xrow[ti].rearrange("p (d e) -> p e d", e=E)[:, e, :]
                        nc.tensor.transpose(
                            ps_t[:, ti * 128 : (ti + 1) * 128], lhs, idn_sb
                        )
                    xe = work.tile([128, GROUP], F32, tag="xpT", bufs=4)
                    nc.vector.tensor_copy(xe, ps_t)
                    # gate: exact fp32, activation-stationary, one PSUM bank
                    for ti in range(NTAU):
                        nc.tensor.matmul(
                            ps_g[:, ti * E : (ti + 1) * E],
                            lhsT=xe[:, ti * 128 : (ti + 1) * 128],
                            rhs=gwp_sb[:, e * E : (e + 1) * E],
                            start=(e == 0 and ti == 0),
                            stop=(e == E - 1 and ti == NTAU - 1),
                            skip_group_check=True,
                        )
                    xer = work.tile([128, GROUP], F32R, tag="xpTr", bufs=3)
                    nc.vector.tensor_copy(xer, xe)
                    xpTr.append(xer)

                    h_sb = work.tile([128, 4 * GROUP], F32R, tag="h", bufs=3)
                    for hq in range(4):
                        ps_fc = psum.tile([128, GROUP], F32, tag="fc", bufs=2)
                        nc.tensor.matmul(
                            ps_fc,
                            lhsT=fcw_sb[:, e * H + hq * 128 : e * H + (hq + 1) * 128],
                            rhs=xer,
                            start=True,
                            stop=True,
                        )
                        nc.scalar.activation(
                            h_sb[:, hq * GROUP : (hq + 1) * GROUP],
                            ps_fc,
                            GELU,
                            bias=fcb_sb[:, e * 4 + hq : e * 4 + hq + 1],
                            scale=1.0,
                        )
                    ps_pj = psum.tile([128, GROUP], F32, tag="pj", bufs=2)
                    for hq in range(4):
                        nc.tensor.matmul(
                            ps_pj,
                            lhsT=pjw_sb[
                                :, (e * 4 + hq) * 128 : (e * 4 + hq + 1) * 128
                            ],
                            rhs=h_sb[:, hq * GROUP : (hq + 1) * GROUP],
                            start=(hq == 0),
                            stop=(hq == 3),
                        )
                    pjT_sb = work.tile([128, GROUP], BF16, tag="pjT", bufs=18)
                    nc.vector.tensor_scalar_add(pjT_sb, ps_pj, pjb_sb[:, e : e + 1])
                    pjT.append(pjT_sb)

                # ---- pass 2: gate threshold ----
                gw = []
                for ti in range(NTAU):
                    gwt = work.tile([128, E], F32, tag="gw", bufs=8)
                    nc.vector.tensor_tensor(
                        gwt, ps_g[:, ti * E : (ti + 1) * E], ngb_sb, ALU.is_gt
                    )
                    gw.append(gwt)

                out_sb = [
                    work.tile([128, C], F32, tag="out", bufs=4, name=f"osb_{g}_{ti}")
                    for ti in range(NTAU)
                ]

                # ---- pass 3: exit transposes + gated evacuation ----
                for e in range(E):
                    ps_o = psum.tile([128, GROUP], BF16, tag="tp", bufs=3)
                    for ti in range(NTAU):
                        nc.tensor.transpose(
                            ps_o[:, ti * 128 : (ti + 1) * 128],
                            pjT[e][:, ti * 128 : (ti + 1) * 128],
                            idnb_sb,
                        )
                    for ti in range(NTAU):
                        nc.vector.tensor_scalar_mul(
                            out_sb[ti][:, e * 128 : (e + 1) * 128],
                            ps_o[:, ti * 128 : (ti + 1) * 128],
                            gw[ti][:, e : e + 1],
                        )

                for ti in range(NTAU):
                    nc.sync.dma_start(
                        out=out_d[t0 + ti * 128 : t0 + (ti + 1) * 128, :],
                        in_=out_sb[ti],
                    )

    nc.compile()
    return nc


def _prep_inputs(x, gate_w, gate_b, fc_w, fc_b, proj_w, proj_b):
    x = np.ascontiguousarray(np.asarray(x, dtype=np.float32)).reshape(NTOK, C)
    gate_w = np.asarray(gate_w, dtype=np.float32)
    gate_b = np.asarray(gate_b, dtype=np.float32)
    fc_w = np.asarray(fc_w, dtype=np.float32)
    fc_b = np.asarray(fc_b, dtype=np.float32)
    proj_w = np.asarray(proj_w, dtype=np.float32)
    proj_b = np.asarray(proj_b, dtype=np.float32)

    # permuted channel order: c' = e*128 + d  ->  orig c = 16*d + e
    cp = np.arange(C)
    orig = 16 * (cp % DE) + cp // DE
    gwp = np.ascontiguousarray(gate_w[:, orig].T)  # [C, E]
    fcw = np.ascontiguousarray(fc_w.transpose(0, 2, 1))  # [E, DE, H]
    pjw = np.ascontiguousarray(
        proj_w.transpose(0, 2, 1).reshape(E, 4, 128, DE)
    )  # [E, q, h_in_chunk, d]
    fcb = np.ascontiguousarray(
        fc_b.reshape(E, 4, 128).transpose(2, 0, 1).reshape(128, E * 4)
    )
    pjb = np.ascontiguousarray(proj_b.T)  # [DE, E]
    ngb = np.ascontiguousarray(np.broadcast_to(-gate_b, (128, E)))
    idn = np.eye(128, dtype=np.float32)
    idnb = np.eye(128, dtype=np.float32).astype(ml_dtypes.bfloat16)

    shared = {
        "gwp": gwp,
        "fcw": fcw,
        "pjw": pjw,
        "fcb": fcb,
        "pjb": pjb,
        "ngb": ngb,
        "idn": idn,
        "idnb": idnb,
    }
    in_maps = [
        {"x": np.ascontiguousarray(x[i * TPC : (i + 1) * TPC]), **shared}
        for i in range(NCORES)
    ]
    return in_maps


def kernel(x, gate_w, gate_b, fc_w, fc_b, proj_w, proj_b, _trace=False, _tmpdir=None):
    if "nc" not in _CACHE:
        _CACHE["nc"] = _build()
    nc = _CACHE["nc"]
    in_maps = _prep_inputs(x, gate_w, gate_b, fc_w, fc_b, proj_w, proj_b)
    res = run_bass_kernel_spmd(
        nc,
        in_maps,
        core_ids=list(range(NCORES)),
        trace=_trace,
        tmpdir=_tmpdir,
    )
    out = np.concatenate([res.results[i]["out"] for i in range(NCORES)], axis=0)
    out = out.reshape(B, T, C)
    if _trace:
        _CACHE["last_result"] = res
    return out


# revision 8
# speedup vs baseline: 1.1230x; 1.0988x over previous
"""DynamicSparseMoE Trainium2 kernel.

Math (per token t):
  logits[e'] = x[t] . gate_w[e'] + gate_b[e']        (C=2048 contraction)
  gw[e']     = 1.0 if logits[e'] > 0 else 0.0
  expert e input: xe[d] = x[t, 16*d + e]  (d=0..127; expert idx fastest in channel)
  h  = gelu(fc_w[e] @ xe + fc_b[e])                   (H=512)
  oe = proj_w[e] @ h + proj_b[e]                      (DE=128)
  out[t, 128*e + d] = gw[e] * oe[d]                   (expert-major output channels)

Strategy: data-parallel over the 16384 tokens across 8 NeuronCores (2048
tokens/core).  Per 512-token group:
  pass1 (per expert): 4 PE transposes of the stride-16 channel slice of the
    row-major x tile -> xe^T [de, tok] fp32; DVE evacuation; 4 exact-fp32
    gate matmuls (activation-stationary, slice-accumulated into a single
    PSUM bank); GPSIMD cast to fp32r; fc as fp32r matmuls (weights
    stationary, N=512); gelu+fc_bias fused on ACT writing fp32r; proj as
    fp32r matmuls accumulating K=512; proj_bias fused into the PSUM
    evacuation (bf16 out).
  pass2: gate threshold (is_gt) on DVE.
  pass3 (per expert): 4 bf16 PE exit transposes; gate multiply fused into
    the final PSUM->SBUF copy as a per-token tensor_scalar; contiguous
    row stores.
"""

import sys

for _p in ("/opt/trn_rl_repo", "/root/.axon_site"):
    if _p not in sys.path:
        sys.path.insert(0, _p)

import ml_dtypes
import numpy as np

import concourse.mybir as mybir
from concourse import bacc
from concourse.bass_utils import run_bass_kernel_spmd
from concourse.tile import TileContext


B, T, C, E = 8, 2048, 2048, 16
DE = C // E  # 128
H = 4 * DE  # 512
NCORES = 8
NTOK = B * T  # 16384
TPC = NTOK // NCORES  # tokens per core: 2048
GROUP = 512  # tokens per group
NTAU = GROUP // 128  # 4 token-tiles per group
NGRP = TPC // GROUP  # 4 groups per core

F32 = mybir.dt.float32
F32R = mybir.dt.float32r
BF16 = mybir.dt.bfloat16
AF = mybir.ActivationFunctionType
ALU = mybir.AluOpType
GELU = AF.Gelu

_CACHE = {}


def _build():
    nc = bacc.Bacc(trn_type="TRN2", num_devices=NCORES)

    x_d = nc.dram_tensor("x", [TPC, C], F32, kind="ExternalInput").ap()
    gwp_d = nc.dram_tensor("gwp", [C, E], F32, kind="ExternalInput").ap()
    fcw_d = nc.dram_tensor("fcw", [E, DE, H], F32, kind="ExternalInput").ap()
    pjw_d = nc.dram_tensor("pjw", [E, 4, 128, DE], F32, kind="ExternalInput").ap()
    fcb_d = nc.dram_tensor("fcb", [128, 64], F32, kind="ExternalInput").ap()
    pjb_d = nc.dram_tensor("pjb", [128, E], F32, kind="ExternalInput").ap()
    ngb_d = nc.dram_tensor("ngb", [128, E], F32, kind="ExternalInput").ap()
    idn_d = nc.dram_tensor("idn", [128, 128], F32, kind="ExternalInput").ap()
    idnb_d = nc.dram_tensor("idnb", [128, 128], BF16, kind="ExternalInput").ap()
    out_d = nc.dram_tensor("out", [TPC, C], F32, kind="ExternalOutput").ap()

    with TileContext(nc) as tc:
        with (
            tc.tile_pool(name="wts", bufs=1) as wts,
            tc.tile_pool(name="work", bufs=2) as work,
            tc.tile_pool(name="psum", bufs=2, space="PSUM") as psum,
        ):
            # ---- resident weights ----
            gwp_sb = wts.tile([128, E * E], F32)  # [p, chunk*16+e']
            nc.sync.dma_start(
                out=gwp_sb.rearrange("p (k e) -> p k e", k=E),
                in_=gwp_d.rearrange("(k p) e -> p k e", p=128),
            )
            # fc/proj weights: SWDGE dma with cast fp32 -> fp32r
            fcw_sb = wts.tile([128, E * H], F32R)  # [de, e*512+h]
            nc.gpsimd.dma_start(
                out=fcw_sb.rearrange("p (e h) -> p e h", e=E),
                in_=fcw_d.rearrange("e p h -> p e h"),
            )
            pjw_sb = wts.tile([128, E * 4 * DE], F32R)  # [h_in_chunk, (e*4+q)*128+d]
            nc.gpsimd.dma_start(
                out=pjw_sb.rearrange("p (e q d) -> p e q d", e=E, q=4),
                in_=pjw_d.rearrange("e q p d -> p e q d"),
            )
            fcb_sb = wts.tile([128, 64], F32)
            nc.sync.dma_start(out=fcb_sb, in_=fcb_d)
            pjb_sb = wts.tile([128, E], F32)
            nc.sync.dma_start(out=pjb_sb, in_=pjb_d)
            ngb_sb = wts.tile([128, E], F32)
            nc.sync.dma_start(out=ngb_sb, in_=ngb_d)
            idn_sb = wts.tile([128, 128], F32)
            nc.sync.dma_start(out=idn_sb, in_=idn_d)
            idnb_sb = wts.tile([128, 128], BF16)
            nc.sync.dma_start(out=idnb_sb, in_=idnb_d)

            for g in range(NGRP):
                t0 = g * GROUP
                xrow = []
                for ti in range(NTAU):
                    xt = work.tile([128, C], F32, tag="xrow", bufs=4)
                    nc.sync.dma_start(
                        out=xt, in_=x_d[t0 + ti * 128 : t0 + (ti + 1) * 128, :]
                    )
                    xrow.append(xt)

                ps_g = psum.tile([16, GROUP], F32, tag="gate", bufs=1)
                xpTr = []
                pjT = []
                # ---- pass 1: per-expert transposes, gate, fc, gelu, proj ----
                for e in range(E):
                    ps_t = psum.tile([128, GROUP], F32, tag="tp", bufs=3)
                    for ti in range(NTAU):
                        lhs = xrow[ti].rearrange("p (d e) -> p e d", e=E)[:, e, :]
                        nc.tensor.transpose(
                            ps_t[:, ti * 128 : (ti + 1) * 128], lhs, idn_sb
                        )
                    xe = work.tile([128, GROUP], F32, tag="xpT", bufs=4)
                    nc.vector.tensor_copy(xe, ps_t)
                    # gate: exact fp32, weights stationary (tiny LDW), one bank
                    nc.tensor.matmul(
                        ps_g,
                        lhsT=gwp_sb[:, e * E : (e + 1) * E],
                        rhs=xe,
                        start=(e == 0),
                        stop=(e == E - 1),
                    )
                    xer = work.tile([128, GROUP], F32R, tag="xpTr", bufs=3)
                    nc.vector.tensor_copy(xer, xe)
                    xpTr.append(xer)

                    h_sb = work.tile([128, 4 * GROUP], F32R, tag="h", bufs=3)
                    for hq in range(4):
                        ps_fc = psum.tile([128, GROUP], F32, tag="fc", bufs=2)
                        nc.tensor.matmul(
                            ps_fc,
                            lhsT=fcw_sb[:, e * H + hq * 128 : e * H + (hq + 1) * 128],
                            rhs=xer,
                            start=True,
                            stop=True,
                        )
                        nc.scalar.activation(
                            h_sb[:, hq * GROUP : (hq + 1) * GROUP],
                            ps_fc,
                            GELU,
                            bias=fcb_sb[:, e * 4 + hq : e * 4 + hq + 1],
                            scale=1.0,
                        )
                    ps_pj = psum.tile([128, GROUP], F32, tag="pj", bufs=2)
                    for hq in range(4):
                        nc.tensor.matmul(
                            ps_pj,
                            lhsT=pjw_sb[
                                :, (e * 4 + hq) * 128 : (e * 4 + hq + 1) * 128
                            ],
                            rhs=h_sb[:, hq * GROUP : (hq + 1) * GROUP],
                            start=(hq == 0),
                            stop=(hq == 3),
                        )
                    pjT_sb = work.tile([128, GROUP], BF16, tag="pjT", bufs=18)
                    nc.vector.tensor_scalar_add(pjT_sb, ps_pj, pjb_sb[:, e : e + 1])
                    pjT.append(pjT_sb)

                # ---- pass 2: gate evac, transpose to [tok, e], threshold ----
                gsb = work.tile([16, GROUP], F32, tag="gsb", bufs=2)
                nc.vector.tensor_copy(gsb, ps_g)
                ps_gt = psum.tile([128, NTAU * E], F32, tag="tp", bufs=3)
                for ti in range(NTAU):
                    nc.tensor.transpose(
                        ps_gt[:, ti * E : (ti + 1) * E],
                        gsb[:, ti * 128 : (ti + 1) * 128],
                        idn_sb[:16, :16],
                    )
                gw = []
                for ti in range(NTAU):
                    gwt = work.tile([128, E], F32, tag="gw", bufs=8)
                    nc.vector.tensor_tensor(
                        gwt, ps_gt[:, ti * E : (ti + 1) * E], ngb_sb, ALU.is_gt
                    )
                    gw.append(gwt)

                out_sb = [
                    work.tile([128, C], F32, tag="out", bufs=4, name=f"osb_{g}_{ti}")
                    for ti in range(NTAU)
                ]

                # ---- pass 3: exit transposes + gated evacuation ----
                for e in range(E):
                    ps_o = psum.tile([128, GROUP], BF16, tag="tp", bufs=3)
                    for ti in range(NTAU):
                        nc.tensor.transpose(
                            ps_o[:, ti * 128 : (ti + 1) * 128],
                            pjT[e][:, ti * 128 : (ti + 1) * 128],
                            idnb_sb,
                        )
                    for ti in range(NTAU):
                        nc.vector.tensor_scalar_mul(
                            out_sb[ti][:, e * 128 : (e + 1) * 128],
                            ps_o[:, ti * 128 : (ti + 1) * 128],
                            gw[ti][:, e : e + 1],
                        )

                for ti in range(NTAU):
                    nc.sync.dma_start(
                        out=out_d[t0 + ti * 128 : t0 + (ti + 1) * 128, :],
                        in_=out_sb[ti],
                    )

    nc.compile()
    return nc


def _prep_inputs(x, gate_w, gate_b, fc_w, fc_b, proj_w, proj_b):
    x = np.ascontiguousarray(np.asarray(x, dtype=np.float32)).reshape(NTOK, C)
    gate_w = np.asarray(gate_w, dtype=np.float32)
    gate_b = np.asarray(gate_b, dtype=np.float32)
    fc_w = np.asarray(fc_w, dtype=np.float32)
    fc_b = np.asarray(fc_b, dtype=np.float32)
    proj_w = np.asarray(proj_w, dtype=np.float32)
    proj_b = np.asarray(proj_b, dtype=np.float32)

    # permuted channel order: c' = e*128 + d  ->  orig c = 16*d + e
    cp = np.arange(C)
    orig = 16 * (cp % DE) + cp // DE
    gwp = np.ascontiguousarray(gate_w[:, orig].T)  # [C, E]
    fcw = np.ascontiguousarray(fc_w.transpose(0, 2, 1))  # [E, DE, H]
    pjw = np.ascontiguousarray(
        proj_w.transpose(0, 2, 1).reshape(E, 4, 128, DE)
    )  # [E, q, h_in_chunk, d]
    fcb = np.ascontiguousarray(
        fc_b.reshape(E, 4, 128).transpose(2, 0, 1).reshape(128, E * 4)
    )
    pjb = np.ascontiguousarray(proj_b.T)  # [DE, E]
    ngb = np.ascontiguousarray(np.broadcast_to(-gate_b, (128, E)))
    idn = np.eye(128, dtype=np.float32)
    idnb = np.eye(128, dtype=np.float32).astype(ml_dtypes.bfloat16)

    shared = {
        "gwp": gwp,
        "fcw": fcw,
        "pjw": pjw,
        "fcb": fcb,
        "pjb": pjb,
        "ngb": ngb,
        "idn": idn,
        "idnb": idnb,
    }
    in_maps = [
        {"x": np.ascontiguousarray(x[i * TPC : (i + 1) * TPC]), **shared}
        for i in range(NCORES)
    ]
    return in_maps


def kernel(x, gate_w, gate_b, fc_w, fc_b, proj_w, proj_b, _trace=False, _tmpdir=None):
    if "nc" not in _CACHE:
        _CACHE["nc"] = _build()
    nc = _CACHE["nc"]
    in_maps = _prep_inputs(x, gate_w, gate_b, fc_w, fc_b, proj_w, proj_b)
    res = run_bass_kernel_spmd(
        nc,
        in_maps,
        core_ids=list(range(NCORES)),
        trace=_trace,
        tmpdir=_tmpdir,
    )
    out = np.concatenate([res.results[i]["out"] for i in range(NCORES)], axis=0)
    out = out.reshape(B, T, C)
    if _trace:
        _CACHE["last_result"] = res
    return out
